# revision 13
# baseline (speedup 1.0000x reference)
"""Trainium2 Bass kernel for nn_ConstrainedAttentionModel (v2).

Math: with windows[b,t,i,:] = one_hot(x[b,t-i], V) (zero for t-i<0),
q = windows[:, -1], the model is

  scores[b,t] = sum_{i,j} params[i,j] * [ x[b,T-1-i] == x[b,t-j] and t-j>=0 ]
  scores[b,T-1] = -inf
  attn = softmax_t(scores)
  out[b,v] = sum_t attn[b,t] * [x[b,t] == v]

No V-sized einsums: scores come from 9 shifted integer equality-compares
against 3 query scalars, and the output scatter is factorized via
v = 64*hi + lo (hi = x>>6 < 128, lo = x&63 < 64):

  out[hi, lo] = (1/Z) * sum_t onehot128(x_hi[t])[hi] * exp(s_t) * onehot64(x_lo[t])[lo]

as 16 accumulating PE matmuls (t = 16*p + c; chunk c contracts over the
128 partitions p).

v2 engine plan (vs v1's 9449 ns), driven by the TimelineSim cost model:
 - all integer data is int16 end-to-end, params pre-cast to fp16 on the
   host, so no on-chip dtype-convert copies are needed;
 - the 9 score compares collapse into 3 scalar_tensor_tensor ops writing a
   c-major (c,k) block layout whose k=9 column holds the t=T-1 mask,
   prebuilt on Pool during the input-DMA window, so one strided
   tensor_reduce -> exp with fused row-sum gives E directly;
 - the 16 onehot128(xhi) builds fuse into 2 tensor_tensor ops (2x DVE
   mode) over an iota_rep8 constant (col 8h+c -> h) that Pool builds
   during the DMA window; matmul lhsT reads strided views (stride 8);
 - wlo = onehot64(xlo)*E stays per-c tensor_scalar (4x DVE mode, E fused
   free), split DVE/Pool;
 - the output leaves via a kv_writeback SWDGE descriptor PREPARED on Pool
   during the DMA window and fired with trigger_dma right after the Pool
   normalize: the post-compute tail is trigger+transfer+sem (~1.0us)
   instead of the HWDGE path's seq+HWDGE+dge-delay+transfer+sem (~2.4us).

Sharding: pure data parallel, one batch row per NeuronCore (B=8, 8 cores).

Per-core device input inp : int32 (128, 32)  (one DMA):
  cols 0:9    xw int16 (18 vals): xw[p,c] = xpad[16p+c], xpad=[-1,-1,x_b]
  cols 9:11   qi16 int16 (4 vals): [x[T-1], x[T-2], x[T-3], 0]
  cols 11:16  params fp16 (10 halves): params.flat(9) + pad
  cols 16:24  xhi int16 (16 vals): x>>6 per c-chunk
  cols 24:32  xlo int16 (16 vals): x&63
Output out: fp16 (128, 64); out2d[hi, lo] = out[64*hi + lo]; host casts
to f32 and reshapes.
"""

import numpy as np

import concourse.bass as bass
import concourse.tile as tile
from concourse import bacc, mybir
from concourse.bass_utils import run_bass_kernel_spmd

P = 128
T = 2048
NCH = 16          # t = 16*p + c
CB = 8            # c-block for fused hi builds
VHI = 128
VLO = 64
V = VHI * VLO
B = 8
KORD = 3
N_POOL_WLO = 8    # how many wlo chunks build on Pool (rest on DVE)

f32 = mybir.dt.float32
i32 = mybir.dt.int32
i16 = mybir.dt.int16
f16 = mybir.dt.float16
OP = mybir.AluOpType
AF = mybir.ActivationFunctionType

USE_KVWB = False  # kv_writeback trigger path deadlocks TimelineSim template sems


def build_nc():
    nc = bacc.Bacc("TRN2", target_bir_lowering=False, debug=False)

    inp_d = nc.declare_dram_parameter("inp", [P, 41], i32, isOutput=False)
    out_d = nc.declare_dram_parameter("out", [P, VLO], f16, isOutput=True)

    dma_sem = nc.alloc_semaphore("kvwb_dma") if USE_KVWB else None

    with tile.TileContext(nc) as tc:
        with (
            tc.tile_pool(name="const", bufs=1) as cpool,
            tc.tile_pool(name="sb", bufs=1) as spool,
            tc.tile_pool(name="loop", bufs=16) as lpool,
            tc.tile_pool(name="psum", bufs=1, space="PSUM") as ppool,
        ):
            # ---------- DMA-window prebuilds (no input dependency) ----------
            # Final output tile + kv_writeback descriptor prep, first so the
            # SWDGE ring entry exists long before trigger_dma fires it.
            out_sb = spool.tile([P, VLO], f16)
            if USE_KVWB:
                ctx_idx = cpool.tile([P, 1], i32)
                nc.gpsimd.memset(ctx_idx[:], 0)
                in4 = out_sb[:].copy()
                in4.ap = in4.ap[:1] + [[VLO, 1], [VLO, 1], [1, VLO]]
                out4 = out_d[:].copy()
                out4.ap = out4.ap[:0] + [[VHI * VLO, 1], [VLO, VHI], [VLO, 1], [1, VLO]]
                nc.gpsimd.kv_writeback(
                    out_ap=out4, in_ap=in4, ctx_idxs_ap=ctx_idx[:],
                    prepare_only=True, sem=dma_sem)

            # iota_rep8: (128, 1024) int16, col 8h+c -> h.  Slices:
            #  - hi fused builds read it as (p, h, c) 3D views
            #  - matmul lhsT reads [:, c::8]
            #  - plain iota64 for DVE wlo ops comes from [:, 0::8][:, :64]
            iota_rep = cpool.tile([P, VHI * CB], i16)
            nc.gpsimd.iota(iota_rep[:], pattern=[[1, VHI], [0, CB]], base=0,
                           channel_multiplier=0)

            # EQP score blocks (c-major): col 10c+k; k=3i+j<9 from compares,
            # k=9 is the t=T-1 mask column, prebuilt here (data-independent).
            EQP = spool.tile([P, NCH * 10], f16)
            eqp3 = EQP[:].rearrange("p (c k) -> p c k", k=10)
            nc.gpsimd.memset(eqp3[:, :, 9:10], 0.0)
            nc.gpsimd.affine_select(
                out=eqp3[:, :, 9:10], in_=eqp3[:, :, 9:10],
                compare_op=OP.is_ge, fill=-60.0,
                base=T - 2, channel_multiplier=-NCH, pattern=[[-1, NCH], [0, 1]])

            # plain iota64 (col l -> l) for the DVE wlo builds: strided copy
            # out of iota_rep8 runs on otherwise-idle DVE inside the window.
            iota64 = cpool.tile([P, VLO], i16)
            src64 = iota_rep[:, 0:1].copy()
            src64.ap = src64.ap[:-1] + [[CB, VLO]]
            nc.vector.tensor_copy(out=iota64[:], in_=src64)

            ones_col = cpool.tile([P, 1], f32)
            nc.vector.memset(ones_col[:], 1.0)
            ones_row = cpool.tile([1, P], f32)
            nc.vector.memset(ones_row[:], 1.0)

            # ---------- input ----------
            inp_i = spool.tile([P, 41], i32)
            nc.sync.dma_start(out=inp_i[:], in_=inp_d[:])
            xw = inp_i[:, 0:9].bitcast(i16)       # (P, 18)
            qf = inp_i[:, 9:12].bitcast(f32)      # (P, 3): q0 q1 q2
            P9 = inp_i[:, 12:17].bitcast(f16)     # (P, 10): params + pad
            xhi = inp_i[:, 17:25].bitcast(i16)    # (P, 16)
            xlo = inp_i[:, 25:41].bitcast(f32)    # (P, 16)

            # ---------- scores ----------
            # STT i: EQP[p, 10c+3i+j] = (xw[p, 2+c-j] == q_i) * params[i,j]
            for i in range(KORD):
                in0 = xw[:, 2:3].copy()
                in0.ap = in0.ap[:-1] + [[1, NCH], [-1, KORD]]
                nc.vector.scalar_tensor_tensor(
                    out=eqp3[:, :, 3 * i:3 * i + 3],
                    in0=in0,
                    scalar=qf[:, i:i + 1],
                    in1=P9[:, 3 * i:3 * i + 3].unsqueeze(1).broadcast_to(
                        (P, NCH, KORD)),
                    op0=OP.is_equal, op1=OP.mult)

            scores = spool.tile([P, NCH], f16)
            with nc.allow_low_precision(reason="scores: sums of 10 small terms"):
                nc.vector.tensor_reduce(
                    out=scores[:], in_=eqp3,
                    axis=mybir.AxisListType.X, op=OP.add)

            # ---------- E = exp(scores), fused row-sum ----------
            E = spool.tile([P, NCH], f32)
            ssum = spool.tile([P, 1], f32)
            nc.scalar.activation(out=E[:], in_=scores[:], func=AF.Exp,
                                 accum_out=ssum[:])

            # ---------- Z = sum_t exp(s_t) -> rb = broadcast(1/Z) ----------
            tot_p = ppool.tile([1, 1], f32)
            nc.tensor.matmul(out=tot_p[:], lhsT=ones_col[:], rhs=ssum[:],
                             start=True, stop=True)
            rec = spool.tile([1, 1], f32)
            nc.vector.reciprocal(rec[:], tot_p[:])
            rb_p = ppool.tile([P, 1], f32)
            nc.tensor.matmul(out=rb_p[:], lhsT=ones_row[:], rhs=rec[:],
                             start=True, stop=True)
            rb_sb = spool.tile([P, 1], f32)
            nc.vector.tensor_copy(out=rb_sb[:], in_=rb_p[:])

            # ---------- fused onehot128(xhi) builds: 2 tensor_tensor ops ----
            ohhi = spool.tile([P, VHI * NCH], f16)   # col 16h + c
            oh3 = ohhi[:].rearrange("p (h c) -> p h c", c=NCH)
            ir3 = iota_rep[:].rearrange("p (h c) -> p h c", c=CB)
            for g in range(2):
                nc.vector.tensor_tensor(
                    out=oh3[:, :, g * CB:(g + 1) * CB],
                    in0=ir3,
                    in1=xhi[:, g * CB:(g + 1) * CB].unsqueeze(1).broadcast_to(
                        (P, VHI, CB)),
                    op=OP.is_equal)

            # ---------- wlo_c = onehot64(xlo_c) * E_c, then PE matmuls ------
            hi_v = ohhi[:].rearrange("p (h c) -> p c h", c=NCH)
            opsum = ppool.tile([P, VLO], f32)
            for c in range(NCH):
                wlo = lpool.tile([P, VLO], f16, tag="wlo")
                weng = nc.gpsimd if c < N_POOL_WLO else nc.vector
                weng.tensor_scalar(
                    out=wlo[:], in0=iota64[:],
                    scalar1=xlo[:, c:c + 1], scalar2=E[:, c:c + 1],
                    op0=OP.is_equal, op1=OP.mult)
                nc.tensor.matmul(
                    out=opsum[:], lhsT=hi_v[:, c, :], rhs=wlo[:],
                    start=(c == 0), stop=(c == NCH - 1))

            # ---------- normalize on Act (PSUM-capable), fire prepared DMA --
            norm_inst = nc.scalar.activation(
                out=out_sb[:], in_=opsum[:], func=AF.Copy,
                scale=rb_sb[:, 0:1])
            if USE_KVWB:
                from concourse.tile_rust import add_dep_helper
                trig = nc.gpsimd.trigger_dma(count=None)
                add_dep_helper(trig.ins, norm_inst.ins,
                               reason="fire output DMA after normalize")
                # Gate kernel end on DMA completion from SP (idle late, so no
                # ordering cycle with the Pool trigger); clear after the wait
                # so the next launch in-process starts from 0.
                w = nc.sync.wait_ge(dma_sem, 16)
                add_dep_helper(w.ins, trig.ins, reason="wait after trigger")
                cl = nc.sync.sem_clear(dma_sem)
                add_dep_helper(cl.ins, w.ins, reason="clear after wait")
            else:
                nc.sync.dma_start(out=out_d[:], in_=out_sb[:])

    nc.compile()
    return nc


_ROW_IDX = np.arange(P)[:, None] * NCH + np.arange(18)[None, :]  # (128, 18)


def _per_core_inputs(x_b: np.ndarray, params: np.ndarray) -> dict[str, np.ndarray]:
    xpad = np.empty(T + 2, np.int16)
    xpad[:2] = -1
    xpad[2:] = x_b
    inp16 = np.empty((P, 82), np.int16)
    inp16[:, 0:18] = xpad[_ROW_IDX]
    qf = np.array([x_b[T - 1], x_b[T - 2], x_b[T - 3]], np.float32)
    inp16[:, 18:24] = qf.view(np.int16)[None, :]
    p16 = np.zeros(10, np.float16)
    p16[:9] = params.reshape(-1).astype(np.float16)
    inp16[:, 24:34] = p16.view(np.int16)[None, :]
    xmat = inp16[:, 2:18].astype(np.int16)
    inp16[:, 34:50] = xmat >> 6
    inp16[:, 50:82] = (xmat & 63).astype(np.float32).reshape(P, -1).view(np.int16)
    return {"inp": inp16.view(np.int32)}


_NC_CACHE = None


def _get_nc():
    global _NC_CACHE
    if _NC_CACHE is None:
        _NC_CACHE = build_nc()
    return _NC_CACHE


def run(x, params, **spmd_kwargs):
    """Run on 8 NeuronCores; returns (out (8, 8192) fp32, BassKernelResults)."""
    x = np.asarray(x)
    params = np.asarray(params, dtype=np.float32)
    assert x.shape == (B, T), x.shape
    nc = _get_nc()
    in_maps = [_per_core_inputs(x[b].astype(np.int16), params) for b in range(B)]
    res = run_bass_kernel_spmd(nc, in_maps, core_ids=list(range(B)), **spmd_kwargs)
    out = np.stack(
        [res.results[b]["out"].astype(np.float32).reshape(V) for b in range(B)],
        axis=0)
    return np.ascontiguousarray(out), res


def kernel(x, params):
    out, _ = run(x, params)
    return out


# revision 14
# speedup vs baseline: 1.0185x; 1.0185x over previous
"""Trainium2 Bass kernel for nn_ConstrainedAttentionModel (v2).

Math: with windows[b,t,i,:] = one_hot(x[b,t-i], V) (zero for t-i<0),
q = windows[:, -1], the model is

  scores[b,t] = sum_{i,j} params[i,j] * [ x[b,T-1-i] == x[b,t-j] and t-j>=0 ]
  scores[b,T-1] = -inf
  attn = softmax_t(scores)
  out[b,v] = sum_t attn[b,t] * [x[b,t] == v]

No V-sized einsums: scores come from 9 shifted integer equality-compares
against 3 query scalars, and the output scatter is factorized via
v = 64*hi + lo (hi = x>>6 < 128, lo = x&63 < 64):

  out[hi, lo] = (1/Z) * sum_t onehot128(x_hi[t])[hi] * exp(s_t) * onehot64(x_lo[t])[lo]

as 16 accumulating PE matmuls (t = 16*p + c; chunk c contracts over the
128 partitions p).

v2 engine plan (vs v1's 9449 ns), driven by the TimelineSim cost model:
 - all integer data is int16 end-to-end, params pre-cast to fp16 on the
   host, so no on-chip dtype-convert copies are needed;
 - the 9 score compares collapse into 3 scalar_tensor_tensor ops writing a
   c-major (c,k) block layout whose k=9 column holds the t=T-1 mask,
   prebuilt on Pool during the input-DMA window, so one strided
   tensor_reduce -> exp with fused row-sum gives E directly;
 - the 16 onehot128(xhi) builds fuse into 2 tensor_tensor ops (2x DVE
   mode) over an iota_rep8 constant (col 8h+c -> h) that Pool builds
   during the DMA window; matmul lhsT reads strided views (stride 8);
 - wlo = onehot64(xlo)*E stays per-c tensor_scalar (4x DVE mode, E fused
   free), split DVE/Pool;
 - the output leaves via a kv_writeback SWDGE descriptor PREPARED on Pool
   during the DMA window and fired with trigger_dma right after the Pool
   normalize: the post-compute tail is trigger+transfer+sem (~1.0us)
   instead of the HWDGE path's seq+HWDGE+dge-delay+transfer+sem (~2.4us).

Sharding: pure data parallel, one batch row per NeuronCore (B=8, 8 cores).

Per-core device input inp : int32 (128, 32)  (one DMA):
  cols 0:9    xw int16 (18 vals): xw[p,c] = xpad[16p+c], xpad=[-1,-1,x_b]
  cols 9:11   qi16 int16 (4 vals): [x[T-1], x[T-2], x[T-3], 0]
  cols 11:16  params fp16 (10 halves): params.flat(9) + pad
  cols 16:24  xhi int16 (16 vals): x>>6 per c-chunk
  cols 24:32  xlo int16 (16 vals): x&63
Output out: fp16 (128, 64); out2d[hi, lo] = out[64*hi + lo]; host casts
to f32 and reshapes.
"""

import numpy as np

import concourse.bass as bass
import concourse.tile as tile
from concourse import bacc, mybir
from concourse.bass_utils import run_bass_kernel_spmd

P = 128
T = 2048
NCH = 16          # t = 16*p + c
CB = 8            # c-block for fused hi builds
VHI = 128
VLO = 64
V = VHI * VLO
B = 8
KORD = 3
N_POOL_WLO = 7    # how many wlo chunks build on Pool (rest on DVE)

f32 = mybir.dt.float32
i32 = mybir.dt.int32
i16 = mybir.dt.int16
f16 = mybir.dt.float16
OP = mybir.AluOpType
AF = mybir.ActivationFunctionType

USE_KVWB = False  # kv_writeback trigger path deadlocks TimelineSim template sems


def build_nc():
    nc = bacc.Bacc("TRN2", target_bir_lowering=False, debug=False)

    inp_d = nc.declare_dram_parameter("inp", [P, 41], i32, isOutput=False)
    out_d = nc.declare_dram_parameter("out", [P, VLO], f16, isOutput=True)

    dma_sem = nc.alloc_semaphore("kvwb_dma") if USE_KVWB else None

    with tile.TileContext(nc) as tc:
        with (
            tc.tile_pool(name="const", bufs=1) as cpool,
            tc.tile_pool(name="sb", bufs=1) as spool,
            tc.tile_pool(name="loop", bufs=16) as lpool,
            tc.tile_pool(name="psum", bufs=1, space="PSUM") as ppool,
        ):
            # ---------- DMA-window prebuilds (no input dependency) ----------
            # Final output tile + kv_writeback descriptor prep, first so the
            # SWDGE ring entry exists long before trigger_dma fires it.
            out_sb = spool.tile([P, VLO], f16)
            if USE_KVWB:
                ctx_idx = cpool.tile([P, 1], i32)
                nc.gpsimd.memset(ctx_idx[:], 0)
                in4 = out_sb[:].copy()
                in4.ap = in4.ap[:1] + [[VLO, 1], [VLO, 1], [1, VLO]]
                out4 = out_d[:].copy()
                out4.ap = out4.ap[:0] + [[VHI * VLO, 1], [VLO, VHI], [VLO, 1], [1, VLO]]
                nc.gpsimd.kv_writeback(
                    out_ap=out4, in_ap=in4, ctx_idxs_ap=ctx_idx[:],
                    prepare_only=True, sem=dma_sem)

            # iota_rep8: (128, 1024) int16, col 8h+c -> h.  Slices:
            #  - hi fused builds read it as (p, h, c) 3D views
            #  - matmul lhsT reads [:, c::8]
            #  - plain iota64 for DVE wlo ops comes from [:, 0::8][:, :64]
            iota_rep = cpool.tile([P, VHI * CB], i16)
            nc.gpsimd.iota(iota_rep[:], pattern=[[1, VHI], [0, CB]], base=0,
                           channel_multiplier=0)

            # EQP score blocks (c-major): col 10c+k; k=3i+j<9 from compares,
            # k=9 is the t=T-1 mask column, prebuilt here (data-independent).
            EQP = spool.tile([P, NCH * 10], f16)
            eqp3 = EQP[:].rearrange("p (c k) -> p c k", k=10)
            nc.gpsimd.memset(eqp3[:, :, 9:10], 0.0)
            nc.gpsimd.affine_select(
                out=eqp3[:, :, 9:10], in_=eqp3[:, :, 9:10],
                compare_op=OP.is_ge, fill=-60.0,
                base=T - 2, channel_multiplier=-NCH, pattern=[[-1, NCH], [0, 1]])

            # plain iota64 (col l -> l) for the DVE wlo builds: strided copy
            # out of iota_rep8 runs on otherwise-idle DVE inside the window.
            iota64 = cpool.tile([P, VLO], i16)
            src64 = iota_rep[:, 0:1].copy()
            src64.ap = src64.ap[:-1] + [[CB, VLO]]
            nc.vector.tensor_copy(out=iota64[:], in_=src64)

            ones_col = cpool.tile([P, 1], f32)
            nc.vector.memset(ones_col[:], 1.0)
            ones_row = cpool.tile([1, P], f32)
            nc.vector.memset(ones_row[:], 1.0)

            # ---------- input ----------
            inp_i = spool.tile([P, 41], i32)
            nc.sync.dma_start(out=inp_i[:], in_=inp_d[:])
            xw = inp_i[:, 0:9].bitcast(i16)       # (P, 18)
            qf = inp_i[:, 9:12].bitcast(f32)      # (P, 3): q0 q1 q2
            P9 = inp_i[:, 12:17].bitcast(f16)     # (P, 10): params + pad
            xhi = inp_i[:, 17:25].bitcast(i16)    # (P, 16)
            xlo = inp_i[:, 25:41].bitcast(f32)    # (P, 16)

            # ---------- scores ----------
            # STT i: EQP[p, 10c+3i+j] = (xw[p, 2+c-j] == q_i) * params[i,j]
            for i in range(KORD):
                in0 = xw[:, 2:3].copy()
                in0.ap = in0.ap[:-1] + [[1, NCH], [-1, KORD]]
                nc.vector.scalar_tensor_tensor(
                    out=eqp3[:, :, 3 * i:3 * i + 3],
                    in0=in0,
                    scalar=qf[:, i:i + 1],
                    in1=P9[:, 3 * i:3 * i + 3].unsqueeze(1).broadcast_to(
                        (P, NCH, KORD)),
                    op0=OP.is_equal, op1=OP.mult)

            scores = spool.tile([P, NCH], f16)
            with nc.allow_low_precision(reason="scores: sums of 10 small terms"):
                red_inst = nc.vector.tensor_reduce(
                    out=scores[:], in_=eqp3,
                    axis=mybir.AxisListType.X, op=OP.add)

            # ---------- E = exp(scores), fused row-sum ----------
            E = spool.tile([P, NCH], f32)
            ssum = spool.tile([P, 1], f32)
            nc.scalar.activation(out=E[:], in_=scores[:], func=AF.Exp,
                                 accum_out=ssum[:])

            # ---------- Z = sum_t exp(s_t) -> rb = broadcast(1/Z) ----------
            tot_p = ppool.tile([1, 1], f32)
            nc.tensor.matmul(out=tot_p[:], lhsT=ones_col[:], rhs=ssum[:],
                             start=True, stop=True)
            rec = spool.tile([1, 1], f32)
            nc.vector.reciprocal(rec[:], tot_p[:])
            rb_p = ppool.tile([P, 1], f32)
            nc.tensor.matmul(out=rb_p[:], lhsT=ones_row[:], rhs=rec[:],
                             start=True, stop=True)
            rb_sb = spool.tile([P, 1], f32)
            nc.vector.tensor_copy(out=rb_sb[:], in_=rb_p[:])

            # ---------- fused onehot128(xhi) builds: 2 tensor_tensor ops ----
            ohhi = spool.tile([P, VHI * NCH], f16)   # col 16h + c
            oh3 = ohhi[:].rearrange("p (h c) -> p h c", c=NCH)
            ir3 = iota_rep[:].rearrange("p (h c) -> p h c", c=CB)
            from concourse.tile_rust import add_dep_helper
            for g in range(2):
                tt = nc.vector.tensor_tensor(
                    out=oh3[:, :, g * CB:(g + 1) * CB],
                    in0=ir3,
                    in1=xhi[:, g * CB:(g + 1) * CB].unsqueeze(1).broadcast_to(
                        (P, VHI, CB)),
                    op=OP.is_equal)
                if g == 0:
                    # keep DVE clear for the score path: E gates all 16 wlo
                    # builds, so the reduce must not queue behind this 594ns op
                    add_dep_helper(tt.ins, red_inst.ins,
                                   reason="hi builds after score reduce")

            # ---------- wlo_c = onehot64(xlo_c) * E_c, then PE matmuls ------
            hi_v = ohhi[:].rearrange("p (h c) -> p c h", c=NCH)
            opsum = ppool.tile([P, VLO], f32)
            # interleave Pool chunks (start earlier, 184ns each) with DVE
            # chunks (start after the hi builds, 77ns each) in predicted
            # readiness order so the in-order PE queue is never starved.
            pool_cs = list(range(N_POOL_WLO))
            dve_cs = list(range(N_POOL_WLO, NCH))
            order = []
            pt, dt_ = 0.0, 1200.0
    # predicted build-finish times (relative): Pool 184/op, DVE 77/op
            for _ in range(NCH):
                if pool_cs and (not dve_cs or pt + 184 <= dt_ + 77):
                    order.append(('p', pool_cs.pop(0))); pt += 184
                else:
                    order.append(('d', dve_cs.pop(0))); dt_ += 77
            for k, (eng, c) in enumerate(order):
                wlo = lpool.tile([P, VLO], f16, tag="wlo")
                weng = nc.gpsimd if eng == 'p' else nc.vector
                weng.tensor_scalar(
                    out=wlo[:], in0=iota64[:],
                    scalar1=xlo[:, c:c + 1], scalar2=E[:, c:c + 1],
                    op0=OP.is_equal, op1=OP.mult)
                nc.tensor.matmul(
                    out=opsum[:], lhsT=hi_v[:, c, :], rhs=wlo[:],
                    start=(k == 0), stop=(k == NCH - 1))

            # ---------- normalize on Act (PSUM-capable), fire prepared DMA --
            norm_inst = nc.scalar.activation(
                out=out_sb[:], in_=opsum[:], func=AF.Copy,
                scale=rb_sb[:, 0:1])
            if USE_KVWB:
                from concourse.tile_rust import add_dep_helper
                trig = nc.gpsimd.trigger_dma(count=None)
                add_dep_helper(trig.ins, norm_inst.ins,
                               reason="fire output DMA after normalize")
                # Gate kernel end on DMA completion from SP (idle late, so no
                # ordering cycle with the Pool trigger); clear after the wait
                # so the next launch in-process starts from 0.
                w = nc.sync.wait_ge(dma_sem, 16)
                add_dep_helper(w.ins, trig.ins, reason="wait after trigger")
                cl = nc.sync.sem_clear(dma_sem)
                add_dep_helper(cl.ins, w.ins, reason="clear after wait")
            else:
                nc.sync.dma_start(out=out_d[:], in_=out_sb[:])

    nc.compile()
    return nc


_ROW_IDX = np.arange(P)[:, None] * NCH + np.arange(18)[None, :]  # (128, 18)


def _per_core_inputs(x_b: np.ndarray, params: np.ndarray) -> dict[str, np.ndarray]:
    xpad = np.empty(T + 2, np.int16)
    xpad[:2] = -1
    xpad[2:] = x_b
    inp16 = np.empty((P, 82), np.int16)
    inp16[:, 0:18] = xpad[_ROW_IDX]
    qf = np.array([x_b[T - 1], x_b[T - 2], x_b[T - 3]], np.float32)
    inp16[:, 18:24] = qf.view(np.int16)[None, :]
    p16 = np.zeros(10, np.float16)
    p16[:9] = params.reshape(-1).astype(np.float16)
    inp16[:, 24:34] = p16.view(np.int16)[None, :]
    xmat = inp16[:, 2:18].astype(np.int16)
    inp16[:, 34:50] = xmat >> 6
    inp16[:, 50:82] = (xmat & 63).astype(np.float32).reshape(P, -1).view(np.int16)
    return {"inp": inp16.view(np.int32)}


_NC_CACHE = None


def _get_nc():
    global _NC_CACHE
    if _NC_CACHE is None:
        _NC_CACHE = build_nc()
    return _NC_CACHE


def run(x, params, **spmd_kwargs):
    """Run on 8 NeuronCores; returns (out (8, 8192) fp32, BassKernelResults)."""
    x = np.asarray(x)
    params = np.asarray(params, dtype=np.float32)
    assert x.shape == (B, T), x.shape
    nc = _get_nc()
    in_maps = [_per_core_inputs(x[b].astype(np.int16), params) for b in range(B)]
    res = run_bass_kernel_spmd(nc, in_maps, core_ids=list(range(B)), **spmd_kwargs)
    out = np.stack(
        [res.results[b]["out"].astype(np.float32).reshape(V) for b in range(B)],
        axis=0)
    return np.ascontiguousarray(out), res


def kernel(x, params):
    out, _ = run(x, params)
    return out
